# revision 24
# baseline (speedup 1.0000x reference)
"""Trainium2 Bass kernel for additive-attention energies + softmax.

Computes, for hidden [1, B, H], encoder_outputs [T, B, H], W [H, H], b [H]:
    proj[t,b,o]  = sum_h enc[t,b,h] * W[o,h] + b[o]
    energies[b,t] = sum_o hidden[0,b,o] * proj[t,b,o]
    out = softmax(energies, axis=-1)[:, None, :]            # [B, 1, T]

Algebraic rewrite used on-device:
    energies[b,t] = (hidden[b] @ W) . enc[t,b]  +  hidden[b] . b
The second term is constant in t, so it drops out of the softmax entirely.

The dot-product stream runs on the tensor engine with fp16 operands
(host-side cast during sharding; fp32 PSUM accumulation):

  vT[h,b] = sum_o W[o,h] hid[o,b]: W arrives h-chunk-major so each chunk's
            8 accumulating matmuls chase its DMA; vT cast to fp16 vstat.
  E[b,t]  = sum_h vT[h,b] enc[b,h,t]: enc arrives host-transposed as
            [b, hcpair, 128, 2, T] fp16 (512 KB tiles). Per h-chunk round,
            M=1 matmuls (N=512) with stationary vT[:,b] accumulate row b
            into PSUM partition 32*(b%4). Consecutive matmuls rotate the
            tile_position col-group (b 0..3) so each LDWEIGHTS targets an
            idle 32-col sub-array while the previous matmul streams --
            without rotation every LDW+MM pair serializes (~470ns/pair).
  Batches run in two phases (b 0-3 -> PSUM tile E0, b 4-7 -> E1) so E0's
  softmax + output DMAs overlap phase B's stream. Softmax: split max-
  reduce (DVE, th0 half overlaps the last th1 matmuls), Exp w/ bias=-max
  to fp16 (ACT); the host divides by the row sum (softmax is scale-
  invariant). One partition-strided out DMA per phase from the ACT HWDGE
  ring so it doesn't block enc DMAs.

Sharding: data-parallel over batch. Core i handles batches [8i, 8i+8):
  enc slice 16 MB fp16, W replicated 2 MB fp16. No cross-core comm.
Per-core roofline: ~18 MB of HBM reads at ~360 GB/s ~= 50 us.
"""

import sys

import numpy as np

for _p in ("/opt/trn_rl_repo",):
    if _p not in sys.path:
        sys.path.insert(0, _p)

T, B, H = 1024, 64, 1024
NCORES = 8
BPC = B // NCORES  # batches per core
HC = H // 128      # h-chunks (contraction tiles for the energy matmuls)
OC = H // 128      # o-chunks (contraction tiles for the v matmuls)
NP2 = HC // 2      # h-chunk pairs per enc DMA tile
ENC_BUFS = 20

_BASS_CACHE = {}


def _split_multi_waits(nc):
    """This walrus build rejects >1 semaphore wait per instruction for
    several instruction types (Drain/CTRL, LDWEIGHTS, ...). Normalize every
    instruction to <=1 wait: hoist extra waits onto fresh single-wait drain
    clones inserted immediately before it on the same engine (engines are
    serial, so semantics are identical)."""
    import copy

    template = None
    for fn in nc.m.functions:
        for bb in fn.blocks:
            for inst in bb.instructions:
                if type(inst).__name__ == "InstDrain":
                    template = inst
                    break
            if template is not None:
                break
        if template is not None:
            break
    assert template is not None, "no InstDrain found to use as wait-carrier"

    uid = [0]
    for fn in nc.m.functions:
        for bb in fn.blocks:
            out = []
            changed = False
            for inst in bb.instructions:
                si = inst.sync_info
                if si is not None and si.on_wait and len(si.on_wait) > 1:
                    waits = list(si.on_wait)
                    for w in waits[:-1]:
                        d = copy.deepcopy(template)
                        d.name = f"waitsplit-{uid[0]}"
                        uid[0] += 1
                        d.engine = inst.engine
                        dsi = d.sync_info
                        dsi.on_wait = [w]
                        if dsi.on_update:
                            dsi.on_update = []
                        out.append(d)
                        nc.register_instruction(d, overwrite=True)
                    si.on_wait = [waits[-1]]
                    changed = True
                out.append(inst)
            if changed:
                try:
                    bb.instructions = out
                except Exception:
                    bb.instructions.clear()
                    bb.instructions.extend(out)


def _build_bass():
    """Build the per-core Bass program (same program on all 8 cores)."""
    from contextlib import ExitStack

    import concourse.bass as bass
    import concourse.mybir as mybir
    import concourse.tile as tile

    f16 = mybir.dt.float16
    f32 = mybir.dt.float32
    Alu = mybir.AluOpType

    nc = bass.Bass("TRN2")
    # enc[b, p2, p, e, t] = enc_orig[t, b, (2*p2+e)*128 + p]  (fp16)
    enc_h = nc.dram_tensor("enc", [BPC, NP2, 128, 2, T], f16, kind="ExternalInput")
    # w[p, hc, oc, c] = W[oc*128+p, hc*128+c]  (h-chunk-major chunks)
    w_h = nc.dram_tensor("w", [128, HC, OC, 128], f16, kind="ExternalInput")
    # hid[p, oc, b] = hidden[0, core*BPC+b, oc*128+p]
    hid_h = nc.dram_tensor("hid", [128, OC, BPC], f16, kind="ExternalInput")
    # Unnormalized exp(E - max) rows in fp16; the host divides by the row
    # sum (softmax is scale-invariant, so the device skips reciprocal+mul).
    out_h = nc.dram_tensor("out", [BPC, T], f16, kind="ExternalOutput")

    enc, w, hid, out = enc_h.ap(), w_h.ap(), hid_h.ap(), out_h.ap()

    with tile.TileContext(nc) as tc, ExitStack() as ctx:
        const = ctx.enter_context(tc.tile_pool(name="const", bufs=1))
        wpool = ctx.enter_context(tc.tile_pool(name="wpool", bufs=1))
        encpool = ctx.enter_context(tc.tile_pool(name="encp", bufs=ENC_BUFS))
        smpool = ctx.enter_context(tc.tile_pool(name="sm", bufs=1))
        psw = ctx.enter_context(tc.tile_pool(name="psw", bufs=1, space="PSUM"))
        psv = ctx.enter_context(tc.tile_pool(name="psv", bufs=1, space="PSUM"))
        pse0 = ctx.enter_context(tc.tile_pool(name="pse0", bufs=1, space="PSUM"))
        pse1 = ctx.enter_context(tc.tile_pool(name="pse1", bufs=1, space="PSUM"))

        # Alternate DMA triggers between the two HWDGE rings (SP and ACT
        # sequencers): each trigger costs ~0.6us of sequencer time, so
        # splitting them halves the time until all 16 SDMA engines stream.
        dma_ring = [nc.sync, nc.scalar]
        dma_ct = [0]

        def enc_dma(dst, src):
            dma_ring[dma_ct[0] % 2].dma_start(dst, src)
            dma_ct[0] += 1

        # PE warm-up: junk fp32 matmuls (~3.4us busy) so the HAM un-throttles
        # the PE clock (1.2 -> 2.4 GHz) before the v-matmul chain.
        junk = const.tile([128, 128], f32)
        nc.vector.memset(junk[:], 0.0)
        for wi in range(8):
            pw = psw.tile([128, 128], f32, tag="warm")
            nc.tensor.matmul(pw[:], lhsT=junk[:], rhs=junk[:], start=True, stop=True)

        hid_sb = const.tile([128, OC, BPC], f16)
        nc.sync.dma_start(hid_sb[:], hid[:])

        # Head-start enc tiles ahead of the W chunks: keeps all 16 SDMA
        # engines streaming from the first microsecond (W alone engages only
        # half of them); bufs cover the stash until the E rounds drain it.
        head_tiles = {}
        for p2 in range(2):
            for bl in range(4):
                et = encpool.tile([128, 2, T], f16, tag="enc",
                                  name=f"enc_{bl}_{p2}")
                enc_dma(et[:], enc[bl, p2])
                head_tiles[(bl, p2)] = et

        # Preload the ScalarE activation table (Exp) after the head DMA
        # triggers (still long before the softmax Exp needs it).
        actwarm = const.tile([1, 1], f32)
        nc.vector.memset(actwarm[:], 0.0)
        nc.scalar.activation(actwarm[:], actwarm[:],
                             mybir.ActivationFunctionType.Exp)

        # vT[h, b] = sum_o W[o, h] hid[o, b]. W lands h-chunk-major so each
        # chunk's accumulation chain (one open group in the pv bank at a
        # time) runs as soon as its 256 KB chunk lands; vstat[hc] is ready
        # ~1 us after chunk hc's DMA.
        w_sb = wpool.tile([128, HC, OC, 128], f16)
        pv = psv.tile([128, HC * BPC], f32)
        vstat = const.tile([128, HC, BPC], f16)
        for hc in range(HC):
            nc.sync.dma_start(w_sb[:, hc], w[:, hc])
            for oc in range(OC):
                nc.tensor.matmul(
                    pv[:, hc * BPC:(hc + 1) * BPC],
                    lhsT=w_sb[:, hc, oc, :],
                    rhs=hid_sb[:, oc, :],
                    start=(oc == 0),
                    stop=(oc == OC - 1),
                )
            # All copies on DVE: the ACT sequencer now carries half the enc
            # DMA triggers, and a waiting copy would stall its FIFO behind
            # the v chains, delaying those triggers by ~15us.
            nc.vector.tensor_copy(vstat[:, hc, :], pv[:, hc * BPC:(hc + 1) * BPC])

        pes = [
            pse0.tile([128, T], f32, name="pe0"),
            pse1.tile([128, T], f32, name="pe1"),
        ]
        # Zero the energy tiles once up front (banks are idle then): the
        # softmax reduce/exp read all 128 partitions but the matmuls only
        # write rows {0,32,64,96}.
        for pe_t in pes:
            nc.vector.memset(pe_t[:], 0.0)

        def enc_dmas(gi, pre=None):
            """Issue the 16 enc-tile DMAs (512 KB each) for batch group gi."""
            tiles = dict(pre) if pre else {}
            for p2 in range(NP2):
                for bl in range(4):
                    if (bl, p2) in tiles:
                        continue
                    b = gi * 4 + bl
                    et = encpool.tile([128, 2, T], f16, tag="enc",
                                      name=f"enc_{b}_{p2}")
                    enc_dma(et[:], enc[b, p2])
                    tiles[(bl, p2)] = et
            return tiles

        def phase(gi, tiles):
            """Energy matmuls for batch group gi (4 batches -> pes[gi]).

            Consecutive matmuls rotate bl over the four 32-col groups, so
            each matmul's LDWEIGHTS hits an idle sub-array while the
            previous matmul streams; per-(b,th) chains accumulate over hc
            on disjoint partitions 32*bl (+row b within the group's view).
            """
            pe_t = pes[gi]
            for p2 in range(NP2):
                for e in range(2):
                    hc = 2 * p2 + e
                    for th in range(2):
                        for bl in range(4):
                            b = gi * 4 + bl
                            nc.tensor.matmul(
                                pe_t[32 * bl:32 * bl + 1,
                                     th * 512:(th + 1) * 512],
                                lhsT=vstat[:, hc, b:b + 1],
                                rhs=tiles[(bl, p2)][:, e, th * 512:(th + 1) * 512],
                                start=(hc == 0),
                                stop=(hc == HC - 1),
                                tile_position=(0, 32 * bl),
                            )

        def softmax(gi):
            pe_t = pes[gi]
            # Split the row-max by th-half: the cols-0:512 chains close one
            # matmul group earlier than cols-512:1024, and the two halves
            # live in different PSUM banks, so mx0 overlaps the last th1
            # matmuls (no bank collision).
            mx0 = smpool.tile([128, 1], f32, name=f"mx0_{gi}")
            nc.vector.tensor_reduce(out=mx0[:], in_=pe_t[:, 0:512],
                                    axis=mybir.AxisListType.X, op=Alu.max)
            mx1 = smpool.tile([128, 1], f32, name=f"mx1_{gi}")
            nc.vector.tensor_reduce(out=mx1[:], in_=pe_t[:, 512:1024],
                                    axis=mybir.AxisListType.X, op=Alu.max)
            mx = smpool.tile([128, 1], f32, name=f"mx{gi}")
            nc.vector.tensor_tensor(out=mx[:], in0=mx0[:], in1=mx1[:],
                                    op=Alu.max)
            nmx = smpool.tile([128, 1], f32, name=f"nmx{gi}")
            nc.vector.tensor_scalar_mul(nmx[:], mx[:], -1.0)
            ex = smpool.tile([128, T], f16, name=f"ex{gi}")
            nc.scalar.activation(
                ex[:], pe_t[:], mybir.ActivationFunctionType.Exp,
                bias=nmx[:], scale=1.0,
            )
            # Rows b sit on partitions {0,32,64,96}; one partition-strided
            # DMA writes all four. ACT HWDGE ring so it doesn't queue behind
            # enc DMAs on the sync ring.
            nc.scalar.dma_start(out[gi * 4:(gi + 1) * 4, :], ex[0:128:32, :])

        tiles0 = enc_dmas(0, pre=head_tiles)
        phase(0, tiles0)
        tiles1 = enc_dmas(1)   # phase-B DMA triggers precede softmax-0 deps
        softmax(0)
        phase(1, tiles1)
        softmax(1)

        # Teardown trim: no SWDGE DMAs are used anywhere in this kernel, so
        # the per-range gpsimd dma_reset in the tail's semaphore cleanup is
        # dead weight (~1-3us). sem_clear still runs.
        nc.gpsimd.dma_reset = lambda *a, **k: None

    _split_multi_waits(nc)
    return nc


def _get_bass():
    if "nc" not in _BASS_CACHE:
        _BASS_CACHE["nc"] = _build_bass()
    return _BASS_CACHE["nc"]


def make_in_maps(hidden, encoder_outputs, W, b):
    """Shard full inputs into per-core input maps (host-side layout prep)."""
    hidden = np.asarray(hidden, dtype=np.float32)
    encoder_outputs = np.asarray(encoder_outputs, dtype=np.float32)
    W = np.asarray(W, dtype=np.float32)

    enc16 = encoder_outputs.astype(np.float16)          # [T, B, H]
    # Per-b transposes keep each 2 MB block cache-resident.
    encp = np.empty((B, NP2, 128, 2, T), dtype=np.float16)
    for bb in range(B):
        x = np.ascontiguousarray(enc16[:, bb, :]).T      # [H, T]
        encp[bb] = x.reshape(NP2, 2, 128, T).transpose(0, 2, 1, 3)

    # [128, HC, OC, 128]: W[o, h], o -> (oc, p), h -> (hc, c), h-chunk-major
    w_prep = np.ascontiguousarray(
        W.astype(np.float16).reshape(OC, 128, HC, 128).transpose(1, 2, 0, 3))

    # [128, OC, B]: hidden[0, b, o] -> o on partitions
    hid_all = np.ascontiguousarray(
        hidden[0].astype(np.float16).T.reshape(OC, 128, B).transpose(1, 0, 2))

    in_maps = []
    for i in range(NCORES):
        in_maps.append({
            "enc": encp[i * BPC:(i + 1) * BPC],
            "w": w_prep,
            "hid": np.ascontiguousarray(hid_all[:, :, i * BPC:(i + 1) * BPC]),
        })
    return in_maps


def run_on_hw(in_maps, trace=False):
    from concourse.bass_utils import run_bass_kernel_spmd

    nc = _get_bass()
    return run_bass_kernel_spmd(nc, in_maps, list(range(NCORES)), trace=trace)


def gather_output(res):
    """Per-core unnormalized exp rows -> full [B, 1, T] softmax (f32)."""
    parts = [np.asarray(res.results[i]["out"]) for i in range(NCORES)]
    ex = np.concatenate(parts, axis=0).astype(np.float32)  # [B, T]
    ex /= ex.sum(axis=-1, keepdims=True)
    return ex[:, None, :]


def kernel(hidden, encoder_outputs, W, b):
    in_maps = make_in_maps(hidden, encoder_outputs, W, b)
    res = run_on_hw(in_maps, trace=False)
    return gather_output(res)


# revision 28
# speedup vs baseline: 1.0419x; 1.0419x over previous
"""Trainium2 Bass kernel for additive-attention energies + softmax.

Computes, for hidden [1, B, H], encoder_outputs [T, B, H], W [H, H], b [H]:
    proj[t,b,o]  = sum_h enc[t,b,h] * W[o,h] + b[o]
    energies[b,t] = sum_o hidden[0,b,o] * proj[t,b,o]
    out = softmax(energies, axis=-1)[:, None, :]            # [B, 1, T]

Algebraic rewrite used on-device:
    energies[b,t] = (hidden[b] @ W) . enc[t,b]  +  hidden[b] . b
The second term is constant in t, so it drops out of the softmax entirely.

The dot-product stream runs on the tensor engine with fp16 operands
(host-side cast during sharding; fp32 PSUM accumulation):

  vT[h,b] = sum_o W[o,h] hid[o,b]: W arrives h-chunk-major so each chunk's
            8 accumulating matmuls chase its DMA; vT cast to fp16 vstat.
  E[b,t]  = sum_h vT[h,b] enc[b,h,t]: enc arrives host-transposed as
            [b, hcpair, 128, 2, T] fp16 (512 KB tiles). Per h-chunk round,
            M=1 matmuls (N=512) with stationary vT[:,b] accumulate row b
            into PSUM partition 32*(b%4). Consecutive matmuls rotate the
            tile_position col-group (b 0..3) so each LDWEIGHTS targets an
            idle 32-col sub-array while the previous matmul streams --
            without rotation every LDW+MM pair serializes (~470ns/pair).
  Batches run in two phases (b 0-3 -> PSUM tile E0, b 4-7 -> E1) so E0's
  softmax + output DMAs overlap phase B's stream. Softmax: split max-
  reduce (DVE, th0 half overlaps the last th1 matmuls), Exp w/ bias=-max
  to fp16 (ACT); the host divides by the row sum (softmax is scale-
  invariant). One partition-strided out DMA per phase from the ACT HWDGE
  ring so it doesn't block enc DMAs.

Sharding: data-parallel over batch. Core i handles batches [8i, 8i+8):
  enc slice 16 MB fp16, W replicated 2 MB fp16. No cross-core comm.
Per-core roofline: ~18 MB of HBM reads at ~360 GB/s ~= 50 us.
"""

import sys

import numpy as np

for _p in ("/opt/trn_rl_repo",):
    if _p not in sys.path:
        sys.path.insert(0, _p)

T, B, H = 1024, 64, 1024
NCORES = 8
BPC = B // NCORES  # batches per core
HC = H // 128      # h-chunks (contraction tiles for the energy matmuls)
OC = H // 128      # o-chunks (contraction tiles for the v matmuls)
NP2 = HC // 2      # h-chunk pairs per enc DMA tile
ENC_BUFS = 20

_BASS_CACHE = {}


def _split_multi_waits(nc):
    """This walrus build rejects >1 semaphore wait per instruction for
    several instruction types (Drain/CTRL, LDWEIGHTS, ...). Normalize every
    instruction to <=1 wait: hoist extra waits onto fresh single-wait drain
    clones inserted immediately before it on the same engine (engines are
    serial, so semantics are identical)."""
    import copy

    template = None
    for fn in nc.m.functions:
        for bb in fn.blocks:
            for inst in bb.instructions:
                if type(inst).__name__ == "InstDrain":
                    template = inst
                    break
            if template is not None:
                break
        if template is not None:
            break
    assert template is not None, "no InstDrain found to use as wait-carrier"

    uid = [0]
    for fn in nc.m.functions:
        for bb in fn.blocks:
            out = []
            changed = False
            for inst in bb.instructions:
                si = inst.sync_info
                if si is not None and si.on_wait and len(si.on_wait) > 1:
                    waits = list(si.on_wait)
                    for w in waits[:-1]:
                        d = copy.deepcopy(template)
                        d.name = f"waitsplit-{uid[0]}"
                        uid[0] += 1
                        d.engine = inst.engine
                        dsi = d.sync_info
                        dsi.on_wait = [w]
                        if dsi.on_update:
                            dsi.on_update = []
                        out.append(d)
                        nc.register_instruction(d, overwrite=True)
                    si.on_wait = [waits[-1]]
                    changed = True
                out.append(inst)
            if changed:
                try:
                    bb.instructions = out
                except Exception:
                    bb.instructions.clear()
                    bb.instructions.extend(out)


def _build_bass():
    """Build the per-core Bass program (same program on all 8 cores)."""
    from contextlib import ExitStack

    import concourse.bass as bass
    import concourse.mybir as mybir
    import concourse.tile as tile

    f16 = mybir.dt.float16
    f32 = mybir.dt.float32
    Alu = mybir.AluOpType

    nc = bass.Bass("TRN2")
    # enc[b, p2, p, e, t] = enc_orig[t, b, (2*p2+e)*128 + p]  (fp16)
    enc_h = nc.dram_tensor("enc", [BPC, NP2, 128, 2, T], f16, kind="ExternalInput")
    # w[p, hc, oc, c] = W[oc*128+p, hc*128+c]  (h-chunk-major chunks)
    w_h = nc.dram_tensor("w", [128, HC, OC, 128], f16, kind="ExternalInput")
    # hid[p, oc, b] = hidden[0, core*BPC+b, oc*128+p]
    hid_h = nc.dram_tensor("hid", [128, OC, BPC], f16, kind="ExternalInput")
    # Unnormalized exp(E - max) rows in fp16; the host divides by the row
    # sum (softmax is scale-invariant, so the device skips reciprocal+mul).
    out_h = nc.dram_tensor("out", [BPC, T], f16, kind="ExternalOutput")

    enc, w, hid, out = enc_h.ap(), w_h.ap(), hid_h.ap(), out_h.ap()

    with tile.TileContext(nc) as tc, ExitStack() as ctx:
        const = ctx.enter_context(tc.tile_pool(name="const", bufs=1))
        wpool = ctx.enter_context(tc.tile_pool(name="wpool", bufs=1))
        encpool = ctx.enter_context(tc.tile_pool(name="encp", bufs=ENC_BUFS))
        smpool = ctx.enter_context(tc.tile_pool(name="sm", bufs=1))
        psw = ctx.enter_context(tc.tile_pool(name="psw", bufs=1, space="PSUM"))
        psv = ctx.enter_context(tc.tile_pool(name="psv", bufs=1, space="PSUM"))
        pse0 = ctx.enter_context(tc.tile_pool(name="pse0", bufs=1, space="PSUM"))
        pse1 = ctx.enter_context(tc.tile_pool(name="pse1", bufs=1, space="PSUM"))

        # Preload the ScalarE activation table (Exp) during the preamble so
        # the softmax Exp doesn't eat a ~2.7us ACT_TABLE_LOAD mid-kernel.
        actwarm = const.tile([1, 1], f32)
        nc.vector.memset(actwarm[:], 0.0)
        nc.scalar.activation(actwarm[:], actwarm[:],
                             mybir.ActivationFunctionType.Exp)

        # PE warm-up: junk fp32 matmuls (~3.4us busy) so the HAM un-throttles
        # the PE clock (1.2 -> 2.4 GHz) before the v-matmul chain.
        junk = const.tile([128, 128], f32)
        nc.vector.memset(junk[:], 0.0)
        for wi in range(8):
            pw = psw.tile([128, 128], f32, tag="warm")
            nc.tensor.matmul(pw[:], lhsT=junk[:], rhs=junk[:], start=True, stop=True)

        hid_sb = const.tile([128, OC, BPC], f16)
        nc.sync.dma_start(hid_sb[:], hid[:])

        # Head-start enc tiles ahead of the W chunks: keeps all 16 SDMA
        # engines streaming from the first microsecond (W alone engages only
        # half of them); bufs cover the stash until the E rounds drain it.
        head_tiles = {}
        for p2 in range(2):
            for bl in range(4):
                et = encpool.tile([128, 2, T], f16, tag="enc",
                                  name=f"enc_{bl}_{p2}")
                nc.sync.dma_start(et[:], enc[bl, p2])
                head_tiles[(bl, p2)] = et

        # vT[h, b] = sum_o W[o, h] hid[o, b]. W lands h-chunk-major so each
        # chunk's accumulation chain (one open group in the pv bank at a
        # time) runs as soon as its 256 KB chunk lands; vstat[hc] is ready
        # ~1 us after chunk hc's DMA.
        w_sb = wpool.tile([128, HC, OC, 128], f16)
        pv = psv.tile([128, HC * BPC], f32)
        vstat = const.tile([128, HC, BPC], f16)
        for hc in range(HC):
            nc.sync.dma_start(w_sb[:, hc], w[:, hc])
            for oc in range(OC):
                nc.tensor.matmul(
                    pv[:, hc * BPC:(hc + 1) * BPC],
                    lhsT=w_sb[:, hc, oc, :],
                    rhs=hid_sb[:, oc, :],
                    start=(oc == 0),
                    stop=(oc == OC - 1),
                )
            eng = nc.scalar.copy if hc % 2 == 0 else nc.vector.tensor_copy
            eng(vstat[:, hc, :], pv[:, hc * BPC:(hc + 1) * BPC])

        pes = [
            pse0.tile([128, T], f32, name="pe0"),
            pse1.tile([128, T], f32, name="pe1"),
        ]
        # Zero the energy tiles once up front (banks are idle then): the
        # softmax reduce/exp read all 128 partitions but the matmuls only
        # write rows {0,32,64,96}.
        for pe_t in pes:
            nc.vector.memset(pe_t[:], 0.0)

        def enc_dmas(gi, pre=None):
            """Issue the 16 enc-tile DMAs (512 KB each) for batch group gi."""
            tiles = dict(pre) if pre else {}
            for p2 in range(NP2):
                for bl in range(4):
                    if (bl, p2) in tiles:
                        continue
                    b = gi * 4 + bl
                    et = encpool.tile([128, 2, T], f16, tag="enc",
                                      name=f"enc_{b}_{p2}")
                    nc.sync.dma_start(et[:], enc[b, p2])
                    tiles[(bl, p2)] = et
            return tiles

        def phase(gi, tiles):
            """Energy matmuls for batch group gi (4 batches -> pes[gi]).

            Consecutive matmuls rotate bl over the four 32-col groups, so
            each matmul's LDWEIGHTS hits an idle sub-array while the
            previous matmul streams; per-(b,th) chains accumulate over hc
            on disjoint partitions 32*bl (+row b within the group's view).
            """
            pe_t = pes[gi]
            for p2 in range(NP2):
                for e in range(2):
                    hc = 2 * p2 + e
                    for th in range(2):
                        for bl in range(4):
                            b = gi * 4 + bl
                            nc.tensor.matmul(
                                pe_t[32 * bl:32 * bl + 1,
                                     th * 512:(th + 1) * 512],
                                lhsT=vstat[:, hc, b:b + 1],
                                rhs=tiles[(bl, p2)][:, e, th * 512:(th + 1) * 512],
                                start=(hc == 0),
                                stop=(hc == HC - 1),
                                tile_position=(0, 32 * bl),
                            )

        def softmax(gi):
            pe_t = pes[gi]
            # Split the row-max by th-half: the cols-0:512 chains close one
            # matmul group earlier than cols-512:1024, and the two halves
            # live in different PSUM banks, so mx0 overlaps the last th1
            # matmuls (no bank collision).
            mx0 = smpool.tile([128, 1], f32, name=f"mx0_{gi}")
            nc.vector.tensor_reduce(out=mx0[:], in_=pe_t[:, 0:512],
                                    axis=mybir.AxisListType.X, op=Alu.max)
            mx1 = smpool.tile([128, 1], f32, name=f"mx1_{gi}")
            nc.vector.tensor_reduce(out=mx1[:], in_=pe_t[:, 512:1024],
                                    axis=mybir.AxisListType.X, op=Alu.max)
            mx = smpool.tile([128, 1], f32, name=f"mx{gi}")
            nc.vector.tensor_tensor(out=mx[:], in0=mx0[:], in1=mx1[:],
                                    op=Alu.max)
            nmx = smpool.tile([128, 1], f32, name=f"nmx{gi}")
            nc.vector.tensor_scalar_mul(nmx[:], mx[:], -1.0)
            ex = smpool.tile([128, T], f16, name=f"ex{gi}")
            nc.scalar.activation(
                ex[:], pe_t[:], mybir.ActivationFunctionType.Exp,
                bias=nmx[:], scale=1.0,
            )
            # Rows b sit on partitions {0,32,64,96}; one partition-strided
            # DMA writes all four. ACT HWDGE ring so it doesn't queue behind
            # enc DMAs on the sync ring.
            nc.scalar.dma_start(out[gi * 4:(gi + 1) * 4, :], ex[0:128:32, :])

        tiles0 = enc_dmas(0, pre=head_tiles)
        phase(0, tiles0)
        tiles1 = enc_dmas(1)   # phase-B DMA triggers precede softmax-0 deps
        softmax(0)
        phase(1, tiles1)
        softmax(1)

        # Teardown trim: no SWDGE DMAs are used anywhere in this kernel, so
        # the per-range gpsimd dma_reset in the tail's semaphore cleanup is
        # dead weight (~1-3us). sem_clear still runs.
        nc.gpsimd.dma_reset = lambda *a, **k: None

    _split_multi_waits(nc)
    return nc


def _get_bass():
    if "nc" not in _BASS_CACHE:
        _BASS_CACHE["nc"] = _build_bass()
    return _BASS_CACHE["nc"]


def make_in_maps(hidden, encoder_outputs, W, b):
    """Shard full inputs into per-core input maps (host-side layout prep)."""
    hidden = np.asarray(hidden, dtype=np.float32)
    encoder_outputs = np.asarray(encoder_outputs, dtype=np.float32)
    W = np.asarray(W, dtype=np.float32)

    enc16 = encoder_outputs.astype(np.float16)          # [T, B, H]
    # Per-b transposes keep each 2 MB block cache-resident.
    encp = np.empty((B, NP2, 128, 2, T), dtype=np.float16)
    for bb in range(B):
        x = np.ascontiguousarray(enc16[:, bb, :]).T      # [H, T]
        encp[bb] = x.reshape(NP2, 2, 128, T).transpose(0, 2, 1, 3)

    # [128, HC, OC, 128]: W[o, h], o -> (oc, p), h -> (hc, c), h-chunk-major
    w_prep = np.ascontiguousarray(
        W.astype(np.float16).reshape(OC, 128, HC, 128).transpose(1, 2, 0, 3))

    # [128, OC, B]: hidden[0, b, o] -> o on partitions
    hid_all = np.ascontiguousarray(
        hidden[0].astype(np.float16).T.reshape(OC, 128, B).transpose(1, 0, 2))

    in_maps = []
    for i in range(NCORES):
        in_maps.append({
            "enc": encp[i * BPC:(i + 1) * BPC],
            "w": w_prep,
            "hid": np.ascontiguousarray(hid_all[:, :, i * BPC:(i + 1) * BPC]),
        })
    return in_maps


def run_on_hw(in_maps, trace=False):
    from concourse.bass_utils import run_bass_kernel_spmd

    nc = _get_bass()
    return run_bass_kernel_spmd(nc, in_maps, list(range(NCORES)), trace=trace)


def gather_output(res):
    """Per-core unnormalized exp rows -> full [B, 1, T] softmax (f32)."""
    parts = [np.asarray(res.results[i]["out"]) for i in range(NCORES)]
    ex = np.concatenate(parts, axis=0).astype(np.float32)  # [B, T]
    ex /= ex.sum(axis=-1, keepdims=True)
    return ex[:, None, :]


def kernel(hidden, encoder_outputs, W, b):
    in_maps = make_in_maps(hidden, encoder_outputs, W, b)
    res = run_on_hw(in_maps, trace=False)
    return gather_output(res)
